# revision 36
# baseline (speedup 1.0000x reference)
"""TRN2 Bass kernel for causal multi-head attention with RoPE.

Problem: B=2, S=2048, HID=2048, NH=16, HD=128 (fp32 in/out).
Sharding: 8 cores = 2 (batch) x 4 (head-groups of 4 heads).
Each core computes q/k/v projections for its 4 heads (column-parallel),
RoPE, causal attention, and a row-parallel partial o_proj; the host sums
the 4 partials per batch.

v2 (all-bf16 dataflow, no DRAM spill):
  - x / Wq / Wk / Wv / Wo converted to bf16 on host: halves HBM traffic
    and makes every matmul 1 cycle/row regardless of tile width.
  - QT/KT live in SBUF as bf16 (2.1MB each) — the v1 DRAM spill round
    trip (16.8MB) and the chunk-0 reload stall are gone.
  - Q/K weight tiles double-buffered (v1 had 7 x ~6.5us stalls at head
    boundaries, each also dropping the PE p-state clock).
  - Softmax sums accumulate on the idle Vector engine (per-tile adds into
    an f32 accumulator) with a single ones-column matmul per (chunk,head)
    instead of one per tile: removes ~26us of PE rows + 160 LDWEIGHTS.
"""
import os
import sys

if "/opt/trn_rl_repo" not in sys.path:
    sys.path.insert(0, "/opt/trn_rl_repo")

import numpy as np
import ml_dtypes

import concourse.bass as bass
import concourse.mybir as mybir
import concourse.tile as tile
from concourse import bacc
from concourse import bass_isa
from concourse.bass_utils import run_bass_kernel_spmd
from contextlib import ExitStack

P = 128
B, S, HID, NH = 2, 2048, 2048, 16
HD = HID // NH              # 128
H = 4                       # heads per core
DPC = H * HD                # 512 dims per core
KO = HID // P               # 16 contraction chunks
SC = S // 512               # 4 seq chunks of 512
ST = S // P                 # 16 seq tiles of 128
SCALE = 1.0 / float(np.sqrt(HD))

f32 = mybir.dt.float32
f32r = mybir.dt.float32r
bf16 = mybir.dt.bfloat16
fp16 = mybir.dt.float16

_CACHED_NC = None


def build_nc():
    AF = mybir.ActivationFunctionType
    nc = bacc.Bacc(None, target_bir_lowering=False)

    xt = nc.declare_dram_parameter("xt", [P, KO, S], bf16, isOutput=False)
    wq = nc.declare_dram_parameter("wq", [H, P, KO, HD], bf16, isOutput=False)
    wk = nc.declare_dram_parameter("wk", [H, P, KO, HD], bf16, isOutput=False)
    wv = nc.declare_dram_parameter("wv", [P, KO, DPC], bf16, isOutput=False)
    wo = nc.declare_dram_parameter("wo", [P, H, HID], bf16, isOutput=False)
    cosf = nc.declare_dram_parameter("cosf", [P, S], f32, isOutput=False)
    sinf = nc.declare_dram_parameter("sinf", [P, S], f32, isOutput=False)
    bmask = nc.declare_dram_parameter("bmask", [P, H, 512], fp16, isOutput=False)
    # bf16 partials: host sums the 4 head-group partials in f32
    out_p = nc.declare_dram_parameter("out_p", [S, HID], bf16, isOutput=True)

    out3 = out_p.rearrange("(st p) n -> p st n", p=P)

    with tile.TileContext(nc) as tc:
        with ExitStack() as top:
            vpool = top.enter_context(tc.tile_pool(name="vpool", bufs=1))
            qkres = top.enter_context(tc.tile_pool(name="qkres", bufs=1))
            const = top.enter_context(tc.tile_pool(name="const", bufs=1))

            vsb = vpool.tile([P, ST, H, 128], fp16)
            # SBUF-resident transposed Q/K: [d, h, s] in bf16
            qt_sb = qkres.tile([P, H, S], bf16)
            kt_sb = qkres.tile([P, H, S], bf16)

            zb = const.tile([P, 1], f32)
            nc.vector.memset(zb[:], 0.0)
            # warm the scalar-engine exp table so the first attention tile
            # doesn't eat the ACT_TABLE_LOAD latency
            warm = const.tile([P, 1], fp16)
            nc.scalar.activation(warm[:], zb[:], AF.Exp, bias=zb[:], scale=1.0)
            bmt = const.tile([P, H, 512], fp16)

            # ---------------- Phase P: projections ----------------
            with ExitStack() as ctx:
                xpool = ctx.enter_context(tc.tile_pool(name="xp", bufs=1))
                wvpool = ctx.enter_context(tc.tile_pool(name="wvp", bufs=1))
                pp = ctx.enter_context(tc.tile_pool(name="pp", bufs=4, space="PSUM"))

                # per-chunk x tiles + quarter wv tiles: Tile dependencies are
                # tile-granular, so finer tiles let the first V matmuls start
                # after ~2.6MB instead of after the whole stream. wv quarters
                # land first (0.5MB each), then the x chunk halves.
                xsc = [xpool.tile([P, KO, 512], bf16, tag=f"xs{sc}", name=f"xs{sc}")
                       for sc in range(SC)]
                wvq = [wvpool.tile([P, KO // 4, DPC], bf16, tag=f"wv{j}",
                                   name=f"wv{j}") for j in range(4)]
                cspool = ctx.enter_context(tc.tile_pool(name="cs", bufs=1))
                rtmp = ctx.enter_context(tc.tile_pool(name="rt", bufs=4))
                wpool = ctx.enter_context(tc.tile_pool(name="wqk", bufs=2))
                cosT = cspool.tile([P, S], f32)
                sinT = cspool.tile([P, S], f32)

                # Consumers wait on a per-queue DMA completion watermark
                # taken at their issue point, so a matmul effectively waits
                # for EVERY dma issued before it in program order. Issue only
                # the critical bytes (x chunk 0 + wv, ~2.6MB over all three
                # queues) before the first V block; later chunks are issued
                # between blocks, just ahead of their consumers.
                nc.sync.dma_start(wvq[0][:], wv[:, 0:4])
                nc.scalar.dma_start(wvq[1][:], wv[:, 4:8])
                nc.gpsimd.dma_start(wvq[2][:], wv[:, 8:12])
                nc.sync.dma_start(xsc[0][:, 0:6], xt[:, 0:6, 0:512])
                nc.scalar.dma_start(xsc[0][:, 6:11], xt[:, 6:11, 0:512])
                nc.gpsimd.dma_start(xsc[0][:, 11:16], xt[:, 11:16, 0:512])
                nc.sync.dma_start(wvq[3][:], wv[:, 12:16])

                # V natural layout [s, d]: stationary x tile, moving wv
                # (512-wide => full PE rate)
                def v_block(sc):
                    for st in range(sc * 4, sc * 4 + 4):
                        xc = xsc[st // 4]
                        so = (st % 4) * P
                        ps = pp.tile([P, 512], f32, tag="vproj")
                        for ko in range(KO):
                            wvm = wvq[ko // 4][:, ko % 4]
                            nc.tensor.matmul(
                                ps[:],
                                xc[:, ko, so:so + P],
                                wvm,
                                start=(ko == 0),
                                stop=(ko == KO - 1),
                            )
                        nc.vector.tensor_copy(
                            vsb[:, st],
                            ps.rearrange("p (h d) -> p h d", h=H),
                        )

                for sc in range(SC):
                    if sc + 1 < SC:
                        nsl = slice((sc + 1) * 512, (sc + 2) * 512)
                        nc.sync.dma_start(xsc[sc + 1][:, 0:8], xt[:, 0:8, nsl])
                        nc.scalar.dma_start(xsc[sc + 1][:, 8:16], xt[:, 8:16, nsl])
                    else:
                        # full-height tables: cos duplicated halves; sin
                        # signed (-sin rows 0:64, +sin rows 64:128) so the
                        # combine is one add
                        nc.gpsimd.dma_start(cosT[:], cosf[:])
                        nc.gpsimd.dma_start(sinT[:], sinf[:])
                        nc.gpsimd.dma_start(bmt[:], bmask[:])
                    v_block(sc)

                for w4, dst in ((wq, qt_sb), (wk, kt_sb)):
                    for h in range(H):
                        wt = wpool.tile([P, KO, HD], bf16, tag="w")
                        nc.scalar.dma_start(wt[:], w4[h])
                        for sc in range(SC):
                            ssl = slice(sc * 512, (sc + 1) * 512)
                            ps = pp.tile([P, 512], f32, tag="proj")
                            for ko in range(KO):
                                nc.tensor.matmul(
                                    ps[:],
                                    wt[:, ko],
                                    xsc[sc][:, ko],
                                    start=(ko == 0),
                                    stop=(ko == KO - 1),
                                )
                            # RoPE eviction: partition-shifted reads are
                            # legal only with a PSUM operand, so the two
                            # rotate half-ops read ps directly; the combine
                            # writes bf16 into the resident QT/KT.
                            t0 = rtmp.tile([P, 512], f32, tag="t0")
                            t1 = rtmp.tile([P, 512], f32, tag="t1")
                            nc.vector.tensor_mul(t0[0:64], ps[64:128], sinT[0:64, ssl])
                            nc.vector.tensor_mul(t0[64:128], ps[0:64], sinT[64:128, ssl])
                            nc.vector.tensor_mul(t1[:], ps[:], cosT[:, ssl])
                            nc.vector.tensor_add(dst[:, h, ssl], t1[:], t0[:])

            # ------------- Phase A: attention + interleaved o_proj -------------
            with ExitStack() as ctx:
                ppool = ctx.enter_context(tc.tile_pool(name="ppool", bufs=6))
                smpool = ctx.enter_context(tc.tile_pool(name="smp", bufs=2))
                stage = ctx.enter_context(tc.tile_pool(name="stage", bufs=4))
                aopool = ctx.enter_context(tc.tile_pool(name="ao", bufs=1))
                wopool = ctx.enter_context(tc.tile_pool(name="wop", bufs=1))
                ost = ctx.enter_context(tc.tile_pool(name="ost", bufs=4))
                spsum = ctx.enter_context(tc.tile_pool(name="sps", bufs=3, space="PSUM"))
                opsum = ctx.enter_context(tc.tile_pool(name="ops", bufs=2, space="PSUM"))
                opo = ctx.enter_context(tc.tile_pool(name="opo", bufs=3, space="PSUM"))

                # wot's dma is issued after chunk 0's tiles (it would gate
                # c0's first matmuls via the queue watermark otherwise)
                wot = wopool.tile([P, H, HID], bf16)

                aot_c = [
                    aopool.tile([P, H, 512], bf16, tag=f"aot{c}", name=f"aot{c}")
                    for c in range(SC)
                ]

                def emit_og(cc, st4, nch, ev=None):
                    g = st4 * 4 + nch
                    st = cc * 4 + st4
                    pso = opo.tile([P, 512], f32, tag="po", name="pso")
                    for dc in range(H):
                        nc.tensor.matmul(
                            pso[:],
                            aot_c[cc][:, dc, st4 * P:(st4 + 1) * P],
                            wot[:, dc, nch * 512:(nch + 1) * 512],
                            start=(dc == 0),
                            stop=(dc == H - 1),
                        )
                    # mid-stream evictions on DVE (scalar is exp-bound,
                    # gpsimd can't read PSUM); bf16 out halves the write
                    # stream, spread over 3 queues
                    ob = ost.tile([P, 512], bf16, tag="ob", name="ob")
                    if ev is nc.scalar:
                        nc.scalar.activation(ob[:], pso[:], AF.Copy)
                    else:
                        nc.vector.tensor_copy(ob[:], pso[:])
                    eng = (nc.sync, nc.gpsimd, nc.scalar)[g % 3]
                    eng.dma_start(out3[:, st, nch * 512:(nch + 1) * 512], ob[:])

                # Per-head normalize: the partition sum runs as a gpsimd
                # all-reduce issued the moment the head's last softmax-sum
                # add is in (gpsimd is idle; its waiting blocks nobody). The
                # DVE part (reciprocal + mul) is deferred by ~2 tiles so it
                # is data-ready when it reaches the in-order DVE queue —
                # keeping the tensor engine entirely out of the chain.
                pending = []

                def norm_flush():
                    if not pending:
                        return
                    pc, ph, pob, sums = pending.pop()
                    rcpb = stage.tile([P, 512], f32, tag="rcpb")
                    nc.vector.reciprocal_approx_fast(rcpb[:], sums[:])
                    nc.vector.tensor_mul(aot_c[pc][:, ph], pob[:], rcpb[:])

                # Software pipeline: each tile's P@V trails its scores by two
                # tiles, so the exp (scalar ACT, ~690ns) finishes behind the
                # next tile's scores plus an o_proj filler group and never
                # stalls the in-order tensor queue. o_proj groups of chunk
                # c-1 are spread through chunk c's tile stream as PE filler.
                inflight = []

                def emit_pv(e):
                    ec, eh, eti, ent, et, eoff, eob, esm, ept = e
                    nc.tensor.matmul(
                        eob[:, eoff:512],
                        vsb[:, et, eh],
                        ept[:, eoff:512],
                        start=(eti == 0),
                        stop=(eti == ent - 1),
                    )
                    # softmax-sum partials on DVE: tile 0 is the full-width
                    # r=0 diagonal, so a copy initializes the accumulator
                    if eti == 0:
                        nc.vector.tensor_copy(esm[:], ept[:])
                        norm_flush()  # previous head's inputs are long ready
                    else:
                        nc.vector.tensor_add(
                            esm[:, eoff:512], esm[:, eoff:512], ept[:, eoff:512]
                        )
                    if eti == ent - 1:
                        sums = stage.tile([P, 512], f32, tag="sums")
                        nc.gpsimd.partition_all_reduce(
                            sums[:], esm[:], 128, bass_isa.ReduceOp.add
                        )
                        pending.append((ec, eh, eob, sums))

                for c in range(SC):
                    base = c * 512
                    nt = 4 * (c + 1)
                    groups = ([(c - 1, st4, nch) for st4 in range(4)
                               for nch in range(4)] if c > 0 else [])
                    gi = 0
                    ntiles = H * nt
                    cad = max(2, ntiles // 16)
                    tcount = 0
                    for h in range(H):
                        # attn_outT accumulator [d, sq] and DVE softmax-sum
                        # accumulator [k mod 128, sq]
                        ob_ps = opsum.tile([P, 512], f32, tag="obp", name="obp")
                        smacc = smpool.tile([P, 512], fp16, tag="sma", name="sma")
                        # diagonal tiles first: their exp+mask latency hides
                        # behind this head's dense unmasked tail
                        t_order = list(range(4 * c, nt)) + list(range(0, 4 * c))
                        for ti, t in enumerate(t_order):
                            r = t - 4 * c
                            off = P * max(r, 0)
                            ps = spsum.tile([P, 512], f32, tag="s")
                            nc.tensor.matmul(
                                ps[:, off:512],
                                kt_sb[:, h, t * P:(t + 1) * P],
                                qt_sb[:, h, base + off:base + 512],
                                start=True,
                                stop=True,
                            )
                            pt = ppool.tile([P, 512], fp16, tag="pt")
                            nc.scalar.activation(
                                pt[:, off:512], ps[:, off:512], AF.Exp,
                                bias=zb[:], scale=SCALE,
                            )
                            if r >= 0:
                                nc.vector.tensor_mul(
                                    pt[:, off:512], pt[:, off:512],
                                    bmt[:, r, off:512],
                                )
                            inflight.append(
                                (c, h, ti, nt, t, off, ob_ps, smacc, pt)
                            )
                            if len(inflight) > 2:
                                emit_pv(inflight.pop(0))
                            tcount += 1
                            if (tcount >= 4 and tcount % cad == 0
                                    and gi < len(groups)):
                                emit_og(*groups[gi])
                                gi += 1
                        if c == 0 and h == 0:
                            # issue late so it doesn't gate c0's matmuls
                            nc.gpsimd.dma_start(wot[:], wo[:])
                    while gi < len(groups):
                        emit_og(*groups[gi])
                        gi += 1
                while inflight:
                    emit_pv(inflight.pop(0))
                norm_flush()
                # final o_proj block: exps are done, scalar is free
                for st4 in range(4):
                    for nch in range(4):
                        emit_og(SC - 1, st4, nch, ev=nc.scalar)

    nc.compile()
    return nc


def _host_prep(hidden_states, position_ids, Wq, Wk, Wv, Wo):
    """Build the 8 per-core input maps (bf16 weights/activations)."""
    inv_freq = 1.0 / (10000.0 ** (np.arange(0, HD, 2, dtype=np.float32) / HD))
    t = np.arange(S, dtype=np.float32)
    freqs = np.outer(t, inv_freq).astype(np.float32)  # [S, 64]

    bm = np.empty((P, H, 512), dtype=np.float32)
    i = np.arange(P)[:, None, None]
    r = np.arange(H)[None, :, None]
    j = np.arange(512)[None, None, :]
    bm[:] = np.where(i + P * r <= j, 1.0, 0.0)
    bm = bm.astype(np.float16)

    in_maps = []
    per_batch = []
    for b in range(B):
        xT = np.ascontiguousarray(hidden_states[b].T)  # [HID, S]
        xt_sw = np.ascontiguousarray(
            xT.reshape(KO, P, S).transpose(1, 0, 2)
        ).astype(ml_dtypes.bfloat16)  # [P, KO, S]
        fp = freqs[position_ids[b]]  # [S, 64]
        ch = np.cos(fp).T            # [64, S]
        sh = np.sin(fp).T
        cosf = np.ascontiguousarray(np.concatenate([ch, ch], axis=0))   # [128, S]
        sinf = np.ascontiguousarray(np.concatenate([-sh, sh], axis=0))  # signed
        per_batch.append((xt_sw, cosf, sinf))

    for core in range(8):
        b, hg = core // 4, core % 4
        sl = slice(hg * DPC, (hg + 1) * DPC)
        xt_sw, cosf, sinf = per_batch[b]
        wq_sw = np.ascontiguousarray(
            Wq[sl].T.reshape(KO, P, H, HD).transpose(2, 1, 0, 3)
        ).astype(ml_dtypes.bfloat16)  # [H, P, KO, HD]
        wk_sw = np.ascontiguousarray(
            Wk[sl].T.reshape(KO, P, H, HD).transpose(2, 1, 0, 3)
        ).astype(ml_dtypes.bfloat16)
        wv_sw = np.ascontiguousarray(
            Wv[sl].T.reshape(KO, P, DPC).transpose(1, 0, 2)
        ).astype(ml_dtypes.bfloat16)  # [P, KO, DPC]
        wo_sw = np.ascontiguousarray(
            Wo[:, sl].T.reshape(H, HD, HID).transpose(1, 0, 2)
        ).astype(ml_dtypes.bfloat16)  # [P, H, HID]
        in_maps.append({
            "xt": xt_sw, "wq": wq_sw, "wk": wk_sw, "wv": wv_sw, "wo": wo_sw,
            "cosf": cosf, "sinf": sinf, "bmask": bm,
        })
    return in_maps


def kernel(hidden_states, attention_mask, position_ids, Wq, Wk, Wv, Wo,
           _trace=False, _trace_kwargs=None):
    global _CACHED_NC
    hidden_states = np.asarray(hidden_states, dtype=np.float32)
    position_ids = np.asarray(position_ids)
    Wq, Wk, Wv, Wo = (np.asarray(w, dtype=np.float32) for w in (Wq, Wk, Wv, Wo))

    if _CACHED_NC is None:
        _CACHED_NC = build_nc()
    nc = _CACHED_NC

    in_maps = _host_prep(hidden_states, position_ids, Wq, Wk, Wv, Wo)
    res = run_bass_kernel_spmd(
        nc, in_maps, list(range(8)), trace=_trace, **(_trace_kwargs or {})
    )

    out = np.empty((B, S, HID), dtype=np.float32)
    for b in range(B):
        acc = res.results[b * 4]["out_p"].astype(np.float32)
        for hg in range(1, 4):
            acc = acc + res.results[b * 4 + hg]["out_p"].astype(np.float32)
        out[b] = acc
    if _trace:
        return out, res
    return out
